# revision 18
# baseline (speedup 1.0000x reference)
"""Block-circulant linear layer on 8 Trainium2 NeuronCores.

Math: y[n, (j,b)] = sum_i circconv(x[n,i,:], c[j,i,:])[b] + bias.
Factorized via packed-real-FFT (halfcomplex, 128 slots of (re,im)):

  stage A (rfft):    t1 = F_pack^T @ x^T     per in-block i, slot-partition out
  permute A->B:      slot-group regroup (4 coalesced DMA row shuffles)
  stage B (mixing):  t2 = W2_g^T @ t1p       block-diagonal per slot-group g
  permute B->C:      slot-major regroup (4 coalesced DMAs)
  stage C (irfft):   y[b, t] = G_slice^T @ t2p  (feature-major out; G stationary)
  bias:              fused into the PSUM->SBUF evacuation (per-partition scalar)

All matmuls bf16 with N=256 moving columns; PSUM fp32.
Sharding: data-parallel, 1024 tokens per core; weights replicated.
Host side: transpose/pack x shards, build F/W2/G/biasfm, unpack y (layout only).
"""

import numpy as np

try:
    import ml_dtypes
    _BF16 = ml_dtypes.bfloat16
except ImportError:  # pragma: no cover
    _BF16 = None

MID_BF16 = True

BLOCK = 256
NB = 16          # in/out blocks
NSLOT = 128      # frequency slots (halfcomplex pairs)
N_CORES = 8
TOK_PER_CORE = 1024
CHUNK = 256      # tokens per pipeline chunk
N_CHUNKS = TOK_PER_CORE // CHUNK
IN_F = NB * BLOCK  # 4096


def _build_weights(c: np.ndarray):
    """Host-side weight construction (float64 for accuracy, cast to f32)."""
    B, K = BLOCK, NSLOT
    b = np.arange(B)
    k = np.arange(K)
    theta = 2 * np.pi * np.outer(b, k) / B
    F_re = np.cos(theta)
    F_im = -np.sin(theta)
    F_im[:, 0] = (-1.0) ** b            # Nyquist column in the c=1 half, k=0
    F_pack = np.concatenate([F_re, F_im], axis=1)   # [256 b, 256 (c,k)]

    G_re = np.zeros((K, B))
    G_im = np.zeros((K, B))
    kk = np.arange(1, K)
    th = 2 * np.pi * np.outer(kk, b) / B
    G_re[1:] = 2.0 * np.cos(th) / B
    G_re[0] = 1.0 / B
    G_im[1:] = -2.0 * np.sin(th) / B
    G_im[0] = ((-1.0) ** b) / B
    G_pack = np.stack([G_re, G_im], axis=0)          # [2, 128 k, 256 b]

    Cf = np.fft.rfft(c.astype(np.float64), axis=-1)  # [j, i, 129]
    A = Cf.real
    Bm = Cf.imag
    W2 = np.zeros((32, 128, 128))
    for g in range(32):
        for s in range(4):
            ks = 4 * g + s
            blk = np.zeros((32, 32))                 # rows (c,i) -> cols (c',j)
            if ks == 0:
                blk[0:16, 0:16] = A[:, :, 0].T
                blk[16:32, 16:32] = A[:, :, 128].T
            else:
                a = A[:, :, ks].T
                bb = Bm[:, :, ks].T
                blk[0:16, 0:16] = a
                blk[16:32, 0:16] = -bb
                blk[0:16, 16:32] = bb
                blk[16:32, 16:32] = a
            W2[g, 32 * s:32 * s + 32, 32 * s:32 * s + 32] = blk

    f_host = (
        F_pack.reshape(2, 128, 2, 128).transpose(1, 0, 2, 3).reshape(128, 512)
    )  # [p=b_local, bh*256 + ch*128 + k]
    w2_host = W2.transpose(1, 0, 2).reshape(128, 32 * 128)   # [p, 128g + m]
    g_host = G_pack.transpose(1, 0, 2).reshape(128, 512)     # [k, ch*256 + b]
    return (
        f_host.astype(np.float32),
        w2_host.astype(np.float32),
        g_host.astype(np.float32),
    )


_NC_CACHE = {}


def _build_module(mid_bf16=True, mid_bufs=2, pin_bufs=2, psum_bufs=(2, 2, 4),
                  perm_engines="3way", io_eng_name="gpsimd", cj_group=4,
                  sw_pipe=True, repeat=1):
    """Build + compile the per-core Bass module (cached)."""
    psum_bufs = tuple(psum_bufs)
    key = ("nc3", mid_bf16, mid_bufs, pin_bufs, psum_bufs, perm_engines,
           io_eng_name, cj_group, sw_pipe, repeat)
    if key in _NC_CACHE:
        return _NC_CACHE[key]

    import concourse.bass as bass  # noqa: F401
    import concourse.mybir as mybir
    import concourse.tile as tile
    from concourse import bacc

    f32 = mybir.dt.float32
    bf16 = mybir.dt.bfloat16
    mid_dt = bf16 if mid_bf16 else f32
    Identity = mybir.ActivationFunctionType.Identity

    nc = bacc.Bacc("TRN2", target_bir_lowering=False, debug=False)

    xt_d = nc.dram_tensor(
        "xt", [N_CHUNKS, 128, 32, CHUNK], mid_dt, kind="ExternalInput"
    )
    f_d = nc.dram_tensor("fw", [128, 512], mid_dt, kind="ExternalInput")
    w2_d = nc.dram_tensor("w2", [128, 4096], mid_dt, kind="ExternalInput")
    g_d = nc.dram_tensor("gw", [128, 512], mid_dt, kind="ExternalInput")
    bias_d = nc.dram_tensor("biasfm", [128, 32], f32, kind="ExternalInput")
    y_d = nc.dram_tensor(
        "y", [N_CHUNKS, 128, 32, CHUNK], mid_dt, kind="ExternalOutput"
    )

    perm_n = [0]

    def perm_eng():
        engs = {
            "ss": (nc.sync, nc.scalar),
            "3way": (nc.sync, nc.scalar, nc.gpsimd),
            "sg": (nc.sync, nc.gpsimd),
            "sgg": (nc.sync, nc.gpsimd, nc.gpsimd),
            "s": (nc.sync,),
            "gp": (nc.gpsimd,),
        }[perm_engines]
        e = engs[perm_n[0] % len(engs)]
        perm_n[0] += 1
        return e

    io_engine = {"gpsimd": "gpsimd", "scalar": "scalar", "sync": "sync"}[io_eng_name]

    with tile.TileContext(nc) as tc:
        with (
            tc.tile_pool(name="wpool", bufs=1) as wpool,
            tc.tile_pool(name="pin", bufs=pin_bufs) as pin,
            tc.tile_pool(name="pt1", bufs=mid_bufs) as pt1,
            tc.tile_pool(name="pt1p", bufs=mid_bufs) as pt1p,
            tc.tile_pool(name="pt2", bufs=mid_bufs) as pt2,
            tc.tile_pool(name="pt2p", bufs=mid_bufs) as pt2p,
            tc.tile_pool(name="pysb", bufs=2) as pysb,
            tc.tile_pool(name="psA", bufs=psum_bufs[0], space="PSUM") as psA,
            tc.tile_pool(name="psB", bufs=psum_bufs[1], space="PSUM") as psB,
            tc.tile_pool(name="psC", bufs=psum_bufs[2], space="PSUM") as psC,
        ):
            f_sb = wpool.tile([128, 512], mid_dt, tag="fw")
            w2_sb = wpool.tile([128, 4096], mid_dt, tag="w2")
            g_sb = wpool.tile([128, 512], mid_dt, tag="gw")
            bias_sb = wpool.tile([128, 32], f32, tag="biasfm")
            nc.sync.dma_start(out=f_sb[:], in_=f_d[:])
            nc.sync.dma_start(out=w2_sb[:], in_=w2_d[:])
            nc.sync.dma_start(out=g_sb[:], in_=g_d[:])
            nc.sync.dma_start(out=bias_sb[:], in_=bias_d[:])

            evac_n = [0]

            def evac(dst, srcp):
                if evac_n[0] % 2 == 0:
                    nc.vector.tensor_copy(dst, srcp)
                else:
                    nc.scalar.copy(dst, srcp)
                evac_n[0] += 1

            def evac_bias(dst, srcp, bias_ap):
                if evac_n[0] % 2 == 0:
                    nc.vector.tensor_scalar_add(dst, srcp, bias_ap)
                else:
                    nc.scalar.activation(dst, srcp, Identity, bias=bias_ap)
                evac_n[0] += 1

            def io(eng_name):
                return getattr(nc, eng_name)

            xts_t = {}
            t1p_t = {}
            t2p_t = {}

            def do_load(ci):
                xts = pin.tile([128, 8192], mid_dt, tag="pin")
                io(io_engine).dma_start(
                    out=xts[:].rearrange("p (f t) -> p f t", f=32),
                    in_=xt_d[ci % N_CHUNKS],
                )
                xts_t[ci] = xts

            def do_A(ci):
                # stage A: rfft per in-block; out t1[k, (16ch+i), t]
                xts = xts_t.pop(ci)
                t1 = pt1.tile([128, 8192], mid_dt, tag="t1")
                for ch in range(2):
                    for i0 in range(0, NB, 2):
                        ps = psA.tile([128, 512], f32, tag="psA")
                        n_mm = 0
                        for bh in range(2):
                            for ii in (i0, i0 + 1):
                                off = (ii - i0) * 256
                                nc.tensor.matmul(
                                    ps[:, off: off + 256],
                                    f_sb[:, bh * 256 + ch * 128:
                                         bh * 256 + ch * 128 + 128],
                                    xts[:, (2 * ii + bh) * 256:
                                        (2 * ii + bh) * 256 + 256],
                                    start=(n_mm == 0),
                                    stop=(n_mm == 3),
                                )
                                n_mm += 1
                        q1 = 16 * ch + i0
                        evac(t1[:, q1 * 256: q1 * 256 + 512], ps[:])
                # permute A->B: t1p[32s+q, g, t] = t1[4g+s, q, t]
                t1p = pt1p.tile([128, 8192], mid_dt, tag="t1p")
                t1v = t1[:].rearrange("p (q t) -> p q t", t=CHUNK)
                for g in range(32):
                    perm_eng().dma_start(
                        out=t1p[:, g * 256: g * 256 + 256],
                        in_=t1v[4 * g: 4 * g + 4],
                    )
                t1p_t[ci] = t1p

            def do_B(ci):
                # stage B: per-slot-group mixing; out t2[(32s+16c+j), g, t]
                t1p = t1p_t.pop(ci)
                t2 = pt2.tile([128, 8192], mid_dt, tag="t2")
                for g0 in range(0, 32, 2):
                    ps = psB.tile([128, 512], f32, tag="psB")
                    for gg in (g0, g0 + 1):
                        off = (gg - g0) * 256
                        nc.tensor.matmul(
                            ps[:, off: off + 256],
                            w2_sb[:, gg * 128: gg * 128 + 128],
                            t1p[:, gg * 256: gg * 256 + 256],
                            start=(gg == g0),
                            stop=(gg == g0 + 1),
                        )
                    evac(t2[:, g0 * 256: g0 * 256 + 512], ps[:])
                # permute B->C: t2p[4g+s, 16c+j, t] = t2[32s+16c+j, g, t]
                t2p = pt2p.tile([128, 8192], mid_dt, tag="t2p")
                t2pv = t2p[:].rearrange("p (q t) -> p q t", t=CHUNK)
                for g in range(32):
                    perm_eng().dma_start(
                        out=t2pv[4 * g: 4 * g + 4],
                        in_=t2[:, g * 256: g * 256 + 256],
                    )
                t2p_t[ci] = t2p

            def do_C(ci):
                # stage C: irfft, feature-major; G slices stationary.
                # ysb[b_local, (2j+bh), t] in two half-tiles (j 0-7, 8-15)
                t2p = t2p_t.pop(ci)
                for half in range(2):
                    ysb = pysb.tile([128, 4096], mid_dt, tag="ysb")
                    for j0 in range(half * 8, half * 8 + 8, cj_group):
                        pss = []
                        for _ in range(cj_group):
                            psc = psC.tile([128, 512], f32, tag="psC")
                            pss.append(psc)
                        for bh in range(2):
                            for ch in range(2):
                                for jj in range(j0, j0 + cj_group):
                                    nc.tensor.matmul(
                                        pss[jj - j0][:, bh * 256: bh * 256 + 256],
                                        g_sb[:, ch * 256 + bh * 128:
                                             ch * 256 + bh * 128 + 128],
                                        t2p[:, (16 * ch + jj) * 256:
                                            (16 * ch + jj) * 256 + 256],
                                        start=(bh == 0 and ch == 0),
                                        stop=(bh == 1 and ch == 1),
                                    )
                        for jj in range(j0, j0 + cj_group):
                            ps = pss[jj - j0]
                            for bh in range(2):
                                q = 2 * jj + bh
                                evac_bias(
                                    ysb[:, (q - half * 16) * 256:
                                        (q - half * 16) * 256 + 256],
                                    ps[:, bh * 256: bh * 256 + 256],
                                    bias_sb[:, q: q + 1],
                                )
                    # store y half-chunk (feature-major packed; host unpacks)
                    io(io_engine).dma_start(
                        out=y_d[ci % N_CHUNKS, :, half * 16: half * 16 + 16],
                        in_=ysb[:].rearrange("p (q t) -> p q t", t=CHUNK),
                    )

            if sw_pipe:
                NT = N_CHUNKS * repeat
                do_load(0)
                if NT > 1:
                    do_load(1)
                for t in range(NT + 2):
                    if t < NT:
                        do_A(t)
                    if t + 2 < NT:
                        do_load(t + 2)
                    if 0 <= t - 1 < NT:
                        do_B(t - 1)
                    if 0 <= t - 2 < NT:
                        do_C(t - 2)
            else:
                do_load(0)
                if N_CHUNKS > 1:
                    do_load(1)
                for t in range(N_CHUNKS):
                    do_A(t)
                    if t + 2 < N_CHUNKS:
                        do_load(t + 2)
                    do_B(t)
                    do_C(t)

    nc.compile()
    _NC_CACHE[key] = nc
    return nc


def prepare_inputs(x, c, bias):
    """Host-side prep: shard + pack x, build weights. Returns per-core in_maps."""
    batch, seq, in_f = x.shape
    n_tok = batch * seq
    xf = np.ascontiguousarray(x.reshape(n_tok, in_f).astype(np.float32))

    f_host, w2_host, g_host = _build_weights(np.asarray(c, dtype=np.float32))
    bias = np.asarray(bias, dtype=np.float32)
    # biasfm[p, 2j+bh] = bias[j*256 + bh*128 + p]
    biasfm = np.ascontiguousarray(
        bias.reshape(NB, 2, 128).transpose(2, 0, 1).reshape(128, 32)
    )
    if MID_BF16:
        f_host = f_host.astype(_BF16)
        w2_host = w2_host.astype(_BF16)
        g_host = g_host.astype(_BF16)

    in_maps = []
    for core in range(N_CORES):
        shard = xf[core * TOK_PER_CORE:(core + 1) * TOK_PER_CORE]  # [1024, 4096]
        # xt[ci, p, f, t] = shard[ci*256 + t, 128*f + p]
        xt = np.ascontiguousarray(
            shard.reshape(N_CHUNKS, CHUNK, 32, 128).transpose(0, 3, 2, 1)
        )
        if MID_BF16:
            xt = xt.astype(_BF16)
        in_maps.append(
            {
                "xt": xt,
                "fw": f_host,
                "w2": w2_host,
                "gw": g_host,
                "biasfm": biasfm,
            }
        )
    return in_maps


def postprocess(y_cores):
    """y_cores: list of per-core y arrays [N_CHUNKS, 128, 32, CHUNK] ->
    full [8192, 4096] float32 (token-major)."""
    outs = []
    for yc in y_cores:
        yc = np.asarray(yc, dtype=np.float32).reshape(N_CHUNKS, 128, NB, 2, CHUNK)
        # y[ci*256+t, j*256+bh*128+p] = yc[ci, p, j, bh, t]
        outs.append(
            yc.transpose(0, 4, 2, 3, 1).reshape(TOK_PER_CORE, IN_F)
        )
    return np.concatenate(outs, axis=0)


def kernel(x: np.ndarray, c: np.ndarray, bias: np.ndarray) -> np.ndarray:
    from concourse.bass_utils import run_bass_kernel_spmd

    batch, seq, in_f = x.shape
    in_maps = prepare_inputs(x, c, bias)
    nc = _build_module(mid_bf16=MID_BF16)
    res = run_bass_kernel_spmd(nc, in_maps, core_ids=list(range(N_CORES)))
    y = postprocess([r["y"] for r in res.results])
    return y.reshape(batch, seq, in_f).astype(x.dtype)


# revision 21
# speedup vs baseline: 3.0751x; 3.0751x over previous
"""Block-circulant linear layer on 8 Trainium2 NeuronCores.

Math: y[n, (j,b)] = sum_i circconv(x[n,i,:], c[j,i,:])[b] + bias.
Factorized via packed-real-FFT (halfcomplex, 128 slots of (re,im)):

  stage A (rfft):    t1 = F_pack^T @ x^T     per in-block i, slot-partition out
  permute A->B:      slot-group regroup (4 coalesced DMA row shuffles)
  stage B (mixing):  t2 = W2_g^T @ t1p       block-diagonal per slot-group g
  permute B->C:      slot-major regroup (4 coalesced DMAs)
  stage C (irfft):   y[b, t] = G_slice^T @ t2p  (feature-major out; G stationary)
  bias:              fused into the PSUM->SBUF evacuation (per-partition scalar)

All matmuls bf16 with N=256 moving columns; PSUM fp32.
Sharding: data-parallel, 1024 tokens per core; weights replicated.
Host side: transpose/pack x shards, build F/W2/G/biasfm, unpack y (layout only).
"""

import numpy as np

try:
    import ml_dtypes
    _BF16 = ml_dtypes.bfloat16
except ImportError:  # pragma: no cover
    _BF16 = None

MID_BF16 = True

BLOCK = 256
NB = 16          # in/out blocks
NSLOT = 128      # frequency slots (halfcomplex pairs)
N_CORES = 8
TOK_PER_CORE = 1024
CHUNK = 256      # tokens per pipeline chunk
N_CHUNKS = TOK_PER_CORE // CHUNK
IN_F = NB * BLOCK  # 4096


def _build_weights(c: np.ndarray):
    """Host-side weight construction (float64 for accuracy, cast to f32)."""
    B, K = BLOCK, NSLOT
    b = np.arange(B)
    k = np.arange(K)
    theta = 2 * np.pi * np.outer(b, k) / B
    F_re = np.cos(theta)
    F_im = -np.sin(theta)
    F_im[:, 0] = (-1.0) ** b            # Nyquist column in the c=1 half, k=0
    F_pack = np.concatenate([F_re, F_im], axis=1)   # [256 b, 256 (c,k)]

    G_re = np.zeros((K, B))
    G_im = np.zeros((K, B))
    kk = np.arange(1, K)
    th = 2 * np.pi * np.outer(kk, b) / B
    G_re[1:] = 2.0 * np.cos(th) / B
    G_re[0] = 1.0 / B
    G_im[1:] = -2.0 * np.sin(th) / B
    G_im[0] = ((-1.0) ** b) / B
    G_pack = np.stack([G_re, G_im], axis=0)          # [2, 128 k, 256 b]

    Cf = np.fft.rfft(c.astype(np.float64), axis=-1)  # [j, i, 129]
    A = Cf.real
    Bm = Cf.imag
    W2 = np.zeros((32, 128, 128))
    for g in range(32):
        for s in range(4):
            ks = 4 * g + s
            blk = np.zeros((32, 32))                 # rows (c,i) -> cols (c',j)
            if ks == 0:
                blk[0:16, 0:16] = A[:, :, 0].T
                blk[16:32, 16:32] = A[:, :, 128].T
            else:
                a = A[:, :, ks].T
                bb = Bm[:, :, ks].T
                blk[0:16, 0:16] = a
                blk[16:32, 0:16] = -bb
                blk[0:16, 16:32] = bb
                blk[16:32, 16:32] = a
            W2[g, 32 * s:32 * s + 32, 32 * s:32 * s + 32] = blk

    f_host = (
        F_pack.reshape(2, 128, 2, 128).transpose(1, 0, 2, 3).reshape(128, 512)
    )  # [p=b_local, bh*256 + ch*128 + k]
    w2_host = W2.transpose(1, 0, 2).reshape(128, 32 * 128)   # [p, 128g + m]
    g_host = G_pack.transpose(1, 0, 2).reshape(128, 512)     # [k, ch*256 + b]
    return (
        f_host.astype(np.float32),
        w2_host.astype(np.float32),
        g_host.astype(np.float32),
    )


_NC_CACHE = {}


def _build_module(mid_bf16=True, mid_bufs=2, pin_bufs=2, psum_bufs=(2, 2, 4),
                  perm_engines="3way", io_eng_name="gpsimd", cj_group=4,
                  sw_pipe=True, repeat=1, skip_perm=False, skip_store=False,
                  skip_mm=False):
    """Build + compile the per-core Bass module (cached)."""
    psum_bufs = tuple(psum_bufs)
    key = ("nc3", mid_bf16, mid_bufs, pin_bufs, psum_bufs, perm_engines,
           io_eng_name, cj_group, sw_pipe, repeat, skip_perm, skip_store,
           skip_mm)
    if key in _NC_CACHE:
        return _NC_CACHE[key]

    import concourse.bass as bass  # noqa: F401
    import concourse.mybir as mybir
    import concourse.tile as tile
    from concourse import bacc

    f32 = mybir.dt.float32
    bf16 = mybir.dt.bfloat16
    mid_dt = bf16 if mid_bf16 else f32
    Identity = mybir.ActivationFunctionType.Identity

    nc = bacc.Bacc("TRN2", target_bir_lowering=False, debug=False)

    xt_d = nc.dram_tensor(
        "xt", [N_CHUNKS, 128, 32, CHUNK], mid_dt, kind="ExternalInput"
    )
    f_d = nc.dram_tensor("fw", [128, 512], mid_dt, kind="ExternalInput")
    w2_d = nc.dram_tensor("w2", [128, 4096], mid_dt, kind="ExternalInput")
    g_d = nc.dram_tensor("gw", [128, 512], mid_dt, kind="ExternalInput")
    bias_d = nc.dram_tensor("biasfm", [128, 32], f32, kind="ExternalInput")
    y_d = nc.dram_tensor(
        "y", [N_CHUNKS, 128, 32, CHUNK], mid_dt, kind="ExternalOutput"
    )

    perm_n = [0]

    def perm_eng():
        engs = {
            "ss": (nc.sync, nc.scalar),
            "3way": (nc.sync, nc.scalar, nc.gpsimd),
            "4way": (nc.sync, nc.scalar, nc.vector, nc.gpsimd),
            "sg": (nc.sync, nc.gpsimd),
            "sgg": (nc.sync, nc.gpsimd, nc.gpsimd),
            "s": (nc.sync,),
            "gp": (nc.gpsimd,),
        }[perm_engines]
        e = engs[perm_n[0] % len(engs)]
        perm_n[0] += 1
        return e

    io_engine = {"gpsimd": "gpsimd", "scalar": "scalar", "sync": "sync"}[io_eng_name]

    with tile.TileContext(nc) as tc:
        with (
            tc.tile_pool(name="wpool", bufs=1) as wpool,
            tc.tile_pool(name="pin", bufs=pin_bufs) as pin,
            tc.tile_pool(name="pt1", bufs=mid_bufs) as pt1,
            tc.tile_pool(name="pt1p", bufs=mid_bufs) as pt1p,
            tc.tile_pool(name="pt2", bufs=mid_bufs) as pt2,
            tc.tile_pool(name="pt2p", bufs=mid_bufs) as pt2p,
            tc.tile_pool(name="pysb", bufs=2) as pysb,
            tc.tile_pool(name="psA", bufs=psum_bufs[0], space="PSUM") as psA,
            tc.tile_pool(name="psB", bufs=psum_bufs[1], space="PSUM") as psB,
            tc.tile_pool(name="psC", bufs=psum_bufs[2], space="PSUM") as psC,
        ):
            f_sb = wpool.tile([128, 512], mid_dt, tag="fw")
            w2_sb = wpool.tile([128, 4096], mid_dt, tag="w2")
            g_sb = wpool.tile([128, 512], mid_dt, tag="gw")
            bias_sb = wpool.tile([128, 32], f32, tag="biasfm")
            nc.sync.dma_start(out=f_sb[:], in_=f_d[:])
            nc.sync.dma_start(out=w2_sb[:], in_=w2_d[:])
            nc.sync.dma_start(out=g_sb[:], in_=g_d[:])
            nc.sync.dma_start(out=bias_sb[:], in_=bias_d[:])

            evac_n = [0]

            def evac(dst, srcp):
                if evac_n[0] % 2 == 0:
                    nc.vector.tensor_copy(dst, srcp)
                else:
                    nc.scalar.copy(dst, srcp)
                evac_n[0] += 1

            def evac_bias(dst, srcp, bias_ap):
                if evac_n[0] % 2 == 0:
                    nc.vector.tensor_scalar_add(dst, srcp, bias_ap)
                else:
                    nc.scalar.activation(dst, srcp, Identity, bias=bias_ap)
                evac_n[0] += 1

            def io(eng_name):
                return getattr(nc, eng_name)

            xts_t = {}
            t1p_t = {}
            t2p_t = {}

            def do_load(ci):
                xts = pin.tile([128, 8192], mid_dt, tag="pin")
                io(io_engine).dma_start(
                    out=xts[:].rearrange("p (f t) -> p f t", f=32),
                    in_=xt_d[ci % N_CHUNKS],
                )
                xts_t[ci] = xts

            def do_A(ci):
                # stage A: rfft per in-block; out t1[k, (16ch+i), t]
                xts = xts_t.pop(ci)
                t1 = pt1.tile([128, 8192], mid_dt, tag="t1")
                for ch in range(2):
                    for i0 in range(0, NB, 2):
                        ps = psA.tile([128, 512], f32, tag="psA")
                        n_mm = 0
                        for bh in range(2):
                            for ii in (i0, i0 + 1):
                                off = (ii - i0) * 256
                                nc.tensor.matmul(
                                    ps[:, off: off + 256],
                                    f_sb[:, bh * 256 + ch * 128:
                                         bh * 256 + ch * 128 + 128],
                                    xts[:, (2 * ii + bh) * 256:
                                        (2 * ii + bh) * 256 + 256],
                                    start=(n_mm == 0),
                                    stop=(n_mm == 3),
                                )
                                n_mm += 1
                        q1 = 16 * ch + i0
                        evac(t1[:, q1 * 256: q1 * 256 + 512], ps[:])
                # permute A->B: t1p[32s+q, g, t] = t1[4g+s, q, t]
                if skip_perm:
                    t1p_t[ci] = t1
                    return
                t1p = pt1p.tile([128, 8192], mid_dt, tag="t1p")
                t1v = t1[:].rearrange("p (q t) -> p q t", t=CHUNK)
                for g in range(32):
                    perm_eng().dma_start(
                        out=t1p[:, g * 256: g * 256 + 256],
                        in_=t1v[4 * g: 4 * g + 4],
                    )
                t1p_t[ci] = t1p

            def do_B(ci):
                # stage B: per-slot-group mixing; out t2[(32s+16c+j), g, t]
                t1p = t1p_t.pop(ci)
                t2 = pt2.tile([128, 8192], mid_dt, tag="t2")
                for g0 in range(0, 32, 2):
                    ps = psB.tile([128, 512], f32, tag="psB")
                    for gg in (g0, g0 + 1):
                        off = (gg - g0) * 256
                        nc.tensor.matmul(
                            ps[:, off: off + 256],
                            w2_sb[:, gg * 128: gg * 128 + 128],
                            t1p[:, gg * 256: gg * 256 + 256],
                            start=(gg == g0),
                            stop=(gg == g0 + 1),
                        )
                    evac(t2[:, g0 * 256: g0 * 256 + 512], ps[:])
                # permute B->C: t2p[4g+s, 16c+j, t] = t2[32s+16c+j, g, t]
                if skip_perm:
                    t2p_t[ci] = t2
                    return
                t2p = pt2p.tile([128, 8192], mid_dt, tag="t2p")
                t2pv = t2p[:].rearrange("p (q t) -> p q t", t=CHUNK)
                for g in range(32):
                    perm_eng().dma_start(
                        out=t2pv[4 * g: 4 * g + 4],
                        in_=t2[:, g * 256: g * 256 + 256],
                    )
                t2p_t[ci] = t2p

            def do_C(ci):
                # stage C: irfft, feature-major; G slices stationary.
                # ysb[b_local, (2j+bh), t] in two half-tiles (j 0-7, 8-15)
                t2p = t2p_t.pop(ci)
                for half in range(2):
                    ysb = pysb.tile([128, 4096], mid_dt, tag="ysb")
                    for j0 in range(half * 8, half * 8 + 8, cj_group):
                        pss = []
                        for _ in range(cj_group):
                            psc = psC.tile([128, 512], f32, tag="psC")
                            pss.append(psc)
                        for bh in range(2):
                            for ch in range(2):
                                for jj in range(j0, j0 + cj_group):
                                    nc.tensor.matmul(
                                        pss[jj - j0][:, bh * 256: bh * 256 + 256],
                                        g_sb[:, ch * 256 + bh * 128:
                                             ch * 256 + bh * 128 + 128],
                                        t2p[:, (16 * ch + jj) * 256:
                                            (16 * ch + jj) * 256 + 256],
                                        start=(bh == 0 and ch == 0),
                                        stop=(bh == 1 and ch == 1),
                                    )
                        for jj in range(j0, j0 + cj_group):
                            ps = pss[jj - j0]
                            for bh in range(2):
                                q = 2 * jj + bh
                                evac_bias(
                                    ysb[:, (q - half * 16) * 256:
                                        (q - half * 16) * 256 + 256],
                                    ps[:, bh * 256: bh * 256 + 256],
                                    bias_sb[:, q: q + 1],
                                )
                    # store y half-chunk (feature-major packed; host unpacks)
                    if skip_store:
                        continue
                    io(io_engine).dma_start(
                        out=y_d[ci % N_CHUNKS, :, half * 16: half * 16 + 16],
                        in_=ysb[:].rearrange("p (q t) -> p q t", t=CHUNK),
                    )

            if sw_pipe:
                NT = N_CHUNKS * repeat
                do_load(0)
                if NT > 1:
                    do_load(1)
                for t in range(NT + 2):
                    if t < NT:
                        do_A(t)
                    if t + 2 < NT:
                        do_load(t + 2)
                    if 0 <= t - 1 < NT:
                        do_B(t - 1)
                    if 0 <= t - 2 < NT:
                        do_C(t - 2)
            else:
                do_load(0)
                if N_CHUNKS > 1:
                    do_load(1)
                for t in range(N_CHUNKS):
                    do_A(t)
                    if t + 2 < N_CHUNKS:
                        do_load(t + 2)
                    do_B(t)
                    do_C(t)

    nc.compile()
    _NC_CACHE[key] = nc
    return nc


def prepare_inputs(x, c, bias):
    """Host-side prep: shard + pack x, build weights. Returns per-core in_maps."""
    batch, seq, in_f = x.shape
    n_tok = batch * seq
    xf = np.ascontiguousarray(x.reshape(n_tok, in_f).astype(np.float32))

    f_host, w2_host, g_host = _build_weights(np.asarray(c, dtype=np.float32))
    bias = np.asarray(bias, dtype=np.float32)
    # biasfm[p, 2j+bh] = bias[j*256 + bh*128 + p]
    biasfm = np.ascontiguousarray(
        bias.reshape(NB, 2, 128).transpose(2, 0, 1).reshape(128, 32)
    )
    if MID_BF16:
        f_host = f_host.astype(_BF16)
        w2_host = w2_host.astype(_BF16)
        g_host = g_host.astype(_BF16)

    in_maps = []
    for core in range(N_CORES):
        shard = xf[core * TOK_PER_CORE:(core + 1) * TOK_PER_CORE]  # [1024, 4096]
        # xt[ci, p, f, t] = shard[ci*256 + t, 128*f + p]
        xt = np.ascontiguousarray(
            shard.reshape(N_CHUNKS, CHUNK, 32, 128).transpose(0, 3, 2, 1)
        )
        if MID_BF16:
            xt = xt.astype(_BF16)
        in_maps.append(
            {
                "xt": xt,
                "fw": f_host,
                "w2": w2_host,
                "gw": g_host,
                "biasfm": biasfm,
            }
        )
    return in_maps


def postprocess(y_cores):
    """y_cores: list of per-core y arrays [N_CHUNKS, 128, 32, CHUNK] ->
    full [8192, 4096] float32 (token-major)."""
    outs = []
    for yc in y_cores:
        yc = np.asarray(yc, dtype=np.float32).reshape(N_CHUNKS, 128, NB, 2, CHUNK)
        # y[ci*256+t, j*256+bh*128+p] = yc[ci, p, j, bh, t]
        outs.append(
            yc.transpose(0, 4, 2, 3, 1).reshape(TOK_PER_CORE, IN_F)
        )
    return np.concatenate(outs, axis=0)


def kernel(x: np.ndarray, c: np.ndarray, bias: np.ndarray) -> np.ndarray:
    from concourse.bass_utils import run_bass_kernel_spmd

    batch, seq, in_f = x.shape
    in_maps = prepare_inputs(x, c, bias)
    nc = _build_module(mid_bf16=MID_BF16)
    res = run_bass_kernel_spmd(nc, in_maps, core_ids=list(range(N_CORES)))
    y = postprocess([r["y"] for r in res.results])
    return y.reshape(batch, seq, in_f).astype(x.dtype)
